# revision 5
# baseline (speedup 1.0000x reference)
"""fp32r upper-triangle Gram; fp8e4 256*(G-thr) output; host band rescue.

Engine budget: PE stream = 38.1us off-diag (224 x [128,512] fp32r
matmuls at 227ns) + 8.3us diag ({512,256}-wide pairs) ~= 46.5us at the
1 cycle/row fp32r roofline; ~12.2us to first real matmul (fixed ~1.3us
preamble + DMA dispatch + ~4.4us DMA latency on chunk 0); ~11us fixed
tail (drain flush + NEFF exit barriers + postamble semaphore storm —
the postamble is invariant, a do-nothing kernel measures 13.8us).

Design:
- Per batch, G = xn^T xn is symmetric: only upper-triangular [128,512]
  tiles are computed; the host mirrors the result. The two cores of a
  batch split each chunk's four 128-row blocks {0,2}/{1,3} via the
  self-inverse in-chunk block perms [0,1,2,3]/[1,0,3,2], which keeps
  the SPMD program identical across cores AND lets the diagonal pair
  be trimmed to widths {512, 256}: perm-row 0 covers the full chunk,
  perm-row 2 only perm-cols 256:512. Every in-chunk pair is still
  covered by one core or the other (2 wasted quarter-tiles vs 6).
- Matmuls are single-pass float32r (PE truncates operands to ~13
  mantissa bits but streams 1 column/cycle). Narrower than ~384 cols
  doesn't pay: LDWEIGHTS (~187ns) stops hiding under the stream.
- The device outputs fp8e4 of 256*(G - thr): sign = adjacency,
  magnitude flags the |G - thr| < 1e-4 band whose entries the host
  recomputes exactly in float64, restoring reference-level accuracy.
- Each tile's PSUM drain runs on ONE engine, alternating Vector/Scalar
  per tile into separate per-engine SBUF buffers (a shared buffer would
  serialize the two engines via whole-tile dependency tracking).
- 7 manually-rotated PSUM tiles + 3-way-rotated output buffers keep
  the distinct-tile count low; output streams in <=2-slot pieces with
  per-slot pieces in the last phase so only ~32KB flushes after the
  final (256-wide, Scalar) drain.
- Input chunks are greedily dispatched full-size (c0,c1 up front, then
  c[m+2] at each phase head): DMA completion has a ~4.4us fixed
  latency and ~180-260 GB/s effective aggregate, so chunk 0 cannot
  land before ~12.2us no matter how it is sliced; chained/serialized
  variants all measured slower. nthr rides the Scalar hwdge queue.
- Garbage-matmul warmup holds the PE HAM clock (1.2->2.4GHz ramp)
  until chunk 0 lands; chunk 0's arrival gates the stream start.
"""

import sys

for _p in ("/opt/trn_rl_repo", "/root/.axon_site/_ro/trn_rl_repo"):
    if _p not in sys.path:
        sys.path.append(_p)

import numpy as np

B, C, N = 4, 384, 4096
HALF = N // 2
KT = C // 128          # 3 contraction tiles
NCHUNK = 8             # 512-wide column chunks
CW = 512
NCORES = 2 * B
PPF_09 = 1.2815515655446004
EPS = 1e-12
MBITS = 13             # fp32r mantissa grid (pre-round on host)
MARGIN = 1e-4          # |G - thr| band recomputed exactly on host
OSCALE = 256.0         # output = fp8e4( OSCALE * (G - thr) )
NWARM = 10             # PE warmup matmuls (run during input DMA wait)

_compiled_nc = None


def _build_nc():
    import concourse.bacc as bacc
    import concourse.tile as tile
    import concourse.mybir as mybir
    from concourse.tile_rust import add_dep_helper

    def _chain(a, b, why):
        # a waits for b's completion (explicit scheduling dependency)
        add_dep_helper(getattr(a, "ins", a), getattr(b, "ins", b),
                       sync=True, reason=why)

    f32 = mybir.dt.float32
    f32r = mybir.dt.float32r
    f8 = mybir.dt.float8e4
    bf16 = mybir.dt.bfloat16
    Alu = mybir.AluOpType
    Act = mybir.ActivationFunctionType

    nc = bacc.Bacc("TRN2", target_bir_lowering=False, debug=False)

    x_d = nc.dram_tensor("xc", [NCHUNK, 128, KT, CW], f32r,
                         kind="ExternalInput")
    nthr_d = nc.dram_tensor("nthr", [128, 1], f32, kind="ExternalInput")
    # [m, row-in-block, rb-slot, col]: partition-major so each output DMA
    # moves multi-KB contiguous runs per partition
    d_d = nc.dram_tensor("d", [NCHUNK, 128, 16, CW], f8,
                         kind="ExternalOutput")

    with tile.TileContext(nc) as tc:
        with tc.tile_pool(name="sb", bufs=1) as sb, \
             tc.tile_pool(name="psum", bufs=1, space="PSUM") as psum:
            nthr_t = sb.tile([128, 1], f32, name="nthr_t")
            dummy = sb.tile([128, CW], bf16, name="dummy")
            nc.vector.memset(dummy[:], 1.0)
            xc = [sb.tile([128, KT, CW], f32r, name=f"xc{c}")
                  for c in range(NCHUNK)]
            pss = [psum.tile([128, CW], f32, name=f"ps{i}") for i in range(7)]
            # 3-way output buffer rotation: phase m's drains WAR-wait on
            # phase (m-3)'s flushes, which gives the (deliberately
            # delayed) flushes plenty of slack
            dvbs = [sb.tile([128, NCHUNK, CW], f8, name=f"dvb{p}")
                    for p in range(3)]
            acbs = [sb.tile([128, NCHUNK, CW], f8, name=f"acb{p}")
                    for p in range(3)]
            indma = [None] * NCHUNK
            indma[0] = nc.sync.dma_start(out=xc[0][:], in_=x_d[0])
            indma[1] = nc.sync.dma_start(out=xc[1][:], in_=x_d[1])
            # nthr is tiny and only needed by the first drain — keep it
            # off the Sync queue entirely so c2/c3 dispatch earlier
            nc.scalar.dma_start(out=nthr_t[:], in_=nthr_d.ap())

            # HAM warmup: garbage matmuls into a scratch bank while the
            # input stream lands; results are never read.
            wps = psum.tile([128, CW], f32, name="wps")
            warm = []
            for i in range(NWARM):
                warm.append(nc.tensor.matmul(wps[:], dummy[:, 0:128],
                                             dummy[:], start=True, stop=True))
            gctr = 0

            # wavefront over moving chunks: tiles for chunk m need only
            # chunks q <= m; chunk m+2's DMA is emitted between phases so
            # input DMAs interleave with output DMAs on the queue.
            # Input discipline (DMA completion has ~4.4us fixed latency,
            # aggregate ~260 GB/s, service biased to older dispatches):
            # c2 free right after c0/c1; c_m (m>=3) chained on c_{m-2} so
            # at most ~2 chunks contend; every output flush of phase m is
            # chained on input c_{m+2} so outputs never steal bandwidth
            # from the input front the stream is about to need. Input
            # dispatches are emitted a phase early (c_{m+3} at phase m's
            # head) to sit ahead of flush dispatches in the Sync queue.
            for m in range(NCHUNK):
                for c in ([2, 3] if m == 0 else
                          [m + 3] if m + 3 < NCHUNK else []):
                    indma[c] = nc.sync.dma_start(out=xc[c][:], in_=x_d[c])
                    if c == 2:
                        # anchor c2's dispatch mid-warmup (~11.4us): late
                        # enough that c0 and c1 each get a clean bandwidth
                        # window, early enough to land before phase 2
                        _chain(indma[c], warm[6], "c2 mid-warmup")
                    else:
                        _chain(indma[c], indma[c - 2],
                               f"input c{c} after c{c - 2}")
                T = 2 * (m + 1)
                # separate per-engine output buffers so the two drain
                # engines never serialize on a shared tile: even rb ->
                # Vector (slots 0..7), odd rb -> Scalar (slots 8..15)
                dvb = dvbs[m % 3]
                acb = acbs[m % 3]
                fgate = indma[m + 2] if m + 2 < NCHUNK else None
                for rb in range(T):
                    q, r = rb // 2, rb % 2
                    # diag trim: the second diagonal tile (perm-row 2 of
                    # chunk m) only needs perm-cols 256:512
                    diag2 = (q == m and r == 1)
                    c0 = 256 if diag2 else 0
                    w = CW - c0
                    ps = pss[gctr % 7]
                    gctr += 1
                    for k in range(KT):
                        nc.tensor.matmul(
                            ps[:, 0:w],
                            xc[q][:, k, 2 * r * 128:(2 * r + 1) * 128],
                            xc[m][:, k, c0:CW],
                            start=(k == 0), stop=(k == KT - 1),
                        )
                    # stream output in <=2-slot pieces as slots complete,
                    # so only a small piece flushes after the last drain
                    sl = rb // 2
                    if m == NCHUNK - 1 and sl >= 6:
                        s0, fire, pw = sl, True, 1    # small final flushes
                    else:
                        s0, fire, pw = sl - 1, sl % 2 == 1, 2
                    if rb % 2 == 0:
                        nc.vector.tensor_scalar(
                            dvb[:, rb // 2, c0:CW], ps[:, 0:w], OSCALE,
                            nthr_t[:], op0=Alu.mult, op1=Alu.add)
                        if fire:
                            fd = nc.sync.dma_start(
                                out=d_d.ap()[m][:, s0:s0 + pw, :],
                                in_=dvb[:, s0:s0 + pw, :])
                            if fgate is not None:
                                _chain(fd, fgate, f"f{m} after c{m + 2}")
                    else:
                        nc.scalar.activation(
                            acb[:, rb // 2, c0:CW], ps[:, 0:w], Act.Identity,
                            bias=nthr_t[:], scale=OSCALE)
                        if fire:
                            if m >= 5:
                                # inputs are done by now: dispatch from the
                                # Scalar queue right behind the drain — the
                                # final flush then needs no cross-queue hop
                                # and no Sync-queue backlog wait
                                if m == NCHUNK - 1 and sl == 7:
                                    # last flush: only cols 256:512 are real
                                    nc.scalar.dma_start(
                                        out=d_d.ap()[m][:, 8 + s0:9 + s0,
                                                        256:],
                                        in_=acb[:, s0:s0 + 1, 256:])
                                else:
                                    nc.scalar.dma_start(
                                        out=d_d.ap()[m][:,
                                                        8 + s0:8 + s0 + pw, :],
                                        in_=acb[:, s0:s0 + pw, :])
                            else:
                                fd = nc.sync.dma_start(
                                    out=d_d.ap()[m][:, 8 + s0:8 + s0 + pw, :],
                                    in_=acb[:, s0:s0 + pw, :])
                                if fgate is not None:
                                    _chain(fd, fgate, f"f{m} after c{m + 2}")
                if m == 0:
                    # clock-hold: re-stream p0's diag1 into the warm bank
                    # (real 512-wide work, never drained). Duration 681ns
                    # <= the minimum observed p0->p1 input wait (0.66us),
                    # so it never delays phase 1 but keeps the HAM clock
                    # from dropping during the wait for chunk 1.
                    for k in range(KT):
                        nc.tensor.matmul(wps[:], xc[0][:, k, 0:128],
                                         xc[0][:, k, :],
                                         start=(k == 0), stop=(k == KT - 1))
                    # second hold (diag2 re-stream, +363ns -> 1.04us total):
                    # still under the typical 1.5-2.5us c1 wait, covering
                    # more of the idle before phase 1
                    for k in range(KT):
                        nc.tensor.matmul(wps[:, 0:256],
                                         xc[0][:, k, 256:384],
                                         xc[0][:, k, 256:512],
                                         start=(k == 0), stop=(k == KT - 1))
                if m < NCHUNK - 1 and m % 2 == 0:
                    # both sides have slots 0..m (odd count): flush slot m
                    fd = nc.sync.dma_start(
                        out=d_d.ap()[m][:, m:m + 1, :], in_=dvb[:, m:m + 1, :])
                    if fgate is not None:
                        _chain(fd, fgate, f"f{m}x after c{m + 2}")
                    fd = nc.sync.dma_start(
                        out=d_d.ap()[m][:, 8 + m:8 + m + 1, :],
                        in_=acb[:, m:m + 1, :])
                    if fgate is not None:
                        _chain(fd, fgate, f"f{m}x after c{m + 2}")
    nc.compile()
    return nc


def get_nc():
    global _compiled_nc
    if _compiled_nc is None:
        _compiled_nc = _build_nc()
    return _compiled_nc


def _round_mant(x, bits):
    """Round fp32 array to `bits` explicit mantissa bits."""
    m, e = np.frexp(x)
    s = np.float32(1 << bits)
    m = np.round(m * s) / s
    return np.ldexp(m, e).astype(np.float32)


# self-inverse in-chunk block perms: core h computes perm-rows {0,2}
# (orig rows {0,2} / {1,3}) of each chunk
_PERM = ([0, 1, 2, 3], [1, 0, 3, 2])

_state = {}


def make_inputs(x):
    xs = np.asarray(x)[:, :, :, 0]                      # (B, C, N) fp32
    nrm = np.sqrt(np.sum(xs * xs, axis=1, keepdims=True))
    xn = xs / np.maximum(nrm, EPS)

    Nsq = float(N) * float(N)
    in_maps = []
    xn64s, thrs = [], []
    for b in range(B):
        xb64 = xn[b].astype(np.float64)
        s = xb64.sum(axis=1)
        M = xb64 @ xb64.T
        sum_g = float(s @ s)
        sum_g2 = float((M * M).sum())
        mean = (2.0 * sum_g - 2.0 * Nsq) / Nsq
        s2 = 4.0 * sum_g2 - 8.0 * sum_g + 4.0 * Nsq
        var = (s2 - Nsq * mean * mean) / (Nsq - 1.0)
        t_b = (mean + PPF_09 * np.sqrt(var) + 2.0) / 2.0
        xn64s.append(xb64)
        thrs.append(t_b)

        nthr_dev = np.full((128, 1), -t_b * OSCALE, np.float32)
        xbr = _round_mant(xn[b].astype(np.float32), MBITS)  # (C, N)
        for h in range(2):
            xloc = xbr.reshape(C, NCHUNK, 4, 128)[:, :, _PERM[h], :]
            xloc = xloc.reshape(C, N)
            xcarr = xloc.reshape(KT, 128, NCHUNK, CW).transpose(2, 1, 0, 3)
            in_maps.append({
                "xc": np.ascontiguousarray(xcarr),
                "nthr": nthr_dev,
            })
    _state["xn64"] = xn64s
    _state["thr"] = thrs
    return in_maps


def assemble(results):
    out = np.empty((2, B * N * N), np.int32)
    iota = np.arange(N, dtype=np.int32)
    neg1 = np.int32(-1)
    for b in range(B):
        dU = np.empty((N, N), np.float32)
        for h in range(2):
            dv = results[2 * b + h]["d"]      # [m, 128, slot, 512] fp8
            for m in range(NCHUNK):
                T = 2 * (m + 1)
                blk = dv[m].astype(np.float32)             # [128, 16, 512]
                if h == 1:
                    blk = blk.reshape(128, 16, 4, 128)[:, :, _PERM[1], :]
                    blk = blk.reshape(128, 16, CW)
                for rb in range(T):
                    q, r = rb // 2, rb % 2
                    a = 4 * q + _PERM[h][2 * r]
                    slot = q + 8 * r
                    # diag trim: perm-cols 256:512 only = orig col-blocks
                    # {2,3} for both perms (perms keep {2,3} in place)
                    cl = 256 if (q == m and r == 1) else 0
                    dU[a * 128:(a + 1) * 128,
                       m * CW + cl:(m + 1) * CW] = blk[:, slot, cl:]
        adjU = (dU > 0).astype(np.uint8)
        nearU = np.triu(np.abs(dU) < MARGIN * OSCALE)
        # mask garbage in never-written regions (below-diag quarters of
        # trimmed diag tiles are inside triu=False anyway, but be safe)
        ii, jj = np.nonzero(nearU)
        if ii.size:
            xn64 = _state["xn64"][b]
            g = np.einsum('ci,ci->i', xn64[:, ii], xn64[:, jj])
            adjU[ii, jj] = g > _state["thr"][b]
        adj = np.triu(adjU)
        adj += np.triu(adjU, 1).T
        src = b * N + iota
        out[0, b * N * N:(b + 1) * N * N] = np.where(
            adj, src[:, None], neg1).ravel()
        out[1, b * N * N:(b + 1) * N * N] = np.where(
            adj, src[None, :], neg1).ravel()
    return out


def kernel(x):
    from concourse.bass_utils import run_bass_kernel_spmd

    nc = get_nc()
    in_maps = make_inputs(x)
    res = run_bass_kernel_spmd(nc, in_maps, list(range(NCORES)))
    return assemble(res.results)


# revision 6
# speedup vs baseline: 1.0298x; 1.0298x over previous
"""fp32r upper-triangle Gram; fp8e4 256*(G-thr) output; host band rescue.

Engine budget: PE stream = 38.1us off-diag (224 x [128,512] fp32r
matmuls at 227ns) + 8.3us diag ({512,256}-wide pairs) ~= 46.5us at the
1 cycle/row fp32r roofline; ~12.2us to first real matmul (fixed ~1.3us
preamble + DMA dispatch + ~4.4us DMA latency on chunk 0); ~11us fixed
tail (drain flush + NEFF exit barriers + postamble semaphore storm —
the postamble is invariant, a do-nothing kernel measures 13.8us).

Design:
- Per batch, G = xn^T xn is symmetric: only upper-triangular [128,512]
  tiles are computed; the host mirrors the result. The two cores of a
  batch split each chunk's four 128-row blocks {0,2}/{1,3} via the
  self-inverse in-chunk block perms [0,1,2,3]/[1,0,3,2], which keeps
  the SPMD program identical across cores AND lets the diagonal pair
  be trimmed to widths {512, 256}: perm-row 0 covers the full chunk,
  perm-row 2 only perm-cols 256:512. Every in-chunk pair is still
  covered by one core or the other (2 wasted quarter-tiles vs 6).
- Matmuls are single-pass float32r (PE truncates operands to ~13
  mantissa bits but streams 1 column/cycle). Narrower than ~384 cols
  doesn't pay: LDWEIGHTS (~187ns) stops hiding under the stream.
- The device outputs fp8e4 of 256*(G - thr): sign = adjacency,
  magnitude flags the |G - thr| < 1e-4 band whose entries the host
  recomputes exactly in float64, restoring reference-level accuracy.
- Each tile's PSUM drain runs on ONE engine, alternating Vector/Scalar
  per tile into separate per-engine SBUF buffers (a shared buffer would
  serialize the two engines via whole-tile dependency tracking).
- 7 manually-rotated PSUM tiles + 3-way-rotated output buffers keep
  the distinct-tile count low; output streams in <=2-slot pieces with
  per-slot pieces in the last phase so only ~32KB flushes after the
  final (256-wide, Scalar) drain.
- Input chunks are greedily dispatched full-size (c0,c1 up front, then
  c[m+2] at each phase head): DMA completion has a ~4.4us fixed
  latency and ~180-260 GB/s effective aggregate, so chunk 0 cannot
  land before ~12.2us no matter how it is sliced; chained/serialized
  variants all measured slower. nthr rides the Scalar hwdge queue.
- Garbage-matmul warmup holds the PE HAM clock (1.2->2.4GHz ramp)
  until chunk 0 lands; chunk 0's arrival gates the stream start.
"""

import sys

for _p in ("/opt/trn_rl_repo", "/root/.axon_site/_ro/trn_rl_repo"):
    if _p not in sys.path:
        sys.path.append(_p)

import numpy as np

B, C, N = 4, 384, 4096
HALF = N // 2
KT = C // 128          # 3 contraction tiles
NCHUNK = 8             # 512-wide column chunks
CW = 512
NCORES = 2 * B
PPF_09 = 1.2815515655446004
EPS = 1e-12
MBITS = 13             # fp32r mantissa grid (pre-round on host)
MARGIN = 1e-4          # |G - thr| band recomputed exactly on host
OSCALE = 256.0         # output = fp8e4( OSCALE * (G - thr) )
NWARM = 10             # PE warmup matmuls (run during input DMA wait)

_compiled_nc = None


def _build_nc():
    import concourse.bacc as bacc
    import concourse.tile as tile
    import concourse.mybir as mybir
    from concourse.tile_rust import add_dep_helper

    def _chain(a, b, why):
        # a waits for b's completion (explicit scheduling dependency)
        add_dep_helper(getattr(a, "ins", a), getattr(b, "ins", b),
                       sync=True, reason=why)

    f32 = mybir.dt.float32
    f32r = mybir.dt.float32r
    f8 = mybir.dt.float8e4
    bf16 = mybir.dt.bfloat16
    Alu = mybir.AluOpType
    Act = mybir.ActivationFunctionType

    nc = bacc.Bacc("TRN2", target_bir_lowering=False, debug=False)

    x_d = nc.dram_tensor("xc", [NCHUNK, 128, KT, CW], f32r,
                         kind="ExternalInput")
    nthr_d = nc.dram_tensor("nthr", [128, 1], f32, kind="ExternalInput")
    # [m, row-in-block, rb-slot, col]: partition-major so each output DMA
    # moves multi-KB contiguous runs per partition
    d_d = nc.dram_tensor("d", [NCHUNK, 128, 16, CW], f8,
                         kind="ExternalOutput")

    with tile.TileContext(nc) as tc:
        with tc.tile_pool(name="sb", bufs=1) as sb, \
             tc.tile_pool(name="psum", bufs=1, space="PSUM") as psum:
            nthr_t = sb.tile([128, 1], f32, name="nthr_t")
            dummy = sb.tile([128, CW], bf16, name="dummy")
            nc.vector.memset(dummy[:], 1.0)
            xc = [sb.tile([128, KT, CW], f32r, name=f"xc{c}")
                  for c in range(NCHUNK)]
            pss = [psum.tile([128, CW], f32, name=f"ps{i}") for i in range(7)]
            # 3-way output buffer rotation: phase m's drains WAR-wait on
            # phase (m-3)'s flushes, which gives the (deliberately
            # delayed) flushes plenty of slack
            dvbs = [sb.tile([128, NCHUNK, CW], f8, name=f"dvb{p}")
                    for p in range(3)]
            acbs = [sb.tile([128, NCHUNK, CW], f8, name=f"acb{p}")
                    for p in range(3)]
            indma = [None] * NCHUNK
            indma[0] = nc.sync.dma_start(out=xc[0][:], in_=x_d[0])
            indma[1] = nc.sync.dma_start(out=xc[1][:], in_=x_d[1])
            # nthr is tiny and only needed by the first drain — keep it
            # off the Sync queue entirely so c2/c3 dispatch earlier
            nc.scalar.dma_start(out=nthr_t[:], in_=nthr_d.ap())

            # HAM warmup: garbage matmuls into a scratch bank while the
            # input stream lands; results are never read.
            wps = psum.tile([128, CW], f32, name="wps")
            warm = []
            for i in range(NWARM):
                warm.append(nc.tensor.matmul(wps[:], dummy[:, 0:128],
                                             dummy[:], start=True, stop=True))
            gctr = 0

            # wavefront over moving chunks: tiles for chunk m need only
            # chunks q <= m; chunk m+2's DMA is emitted between phases so
            # input DMAs interleave with output DMAs on the queue.
            # Input discipline (DMA completion has ~4.4us fixed latency,
            # aggregate ~260 GB/s, service biased to older dispatches):
            # c2 free right after c0/c1; c_m (m>=3) chained on c_{m-2} so
            # at most ~2 chunks contend; every output flush of phase m is
            # chained on input c_{m+2} so outputs never steal bandwidth
            # from the input front the stream is about to need. Input
            # dispatches are emitted a phase early (c_{m+3} at phase m's
            # head) to sit ahead of flush dispatches in the Sync queue.
            for m in range(NCHUNK):
                for c in ([2, 3] if m == 0 else
                          [m + 3] if m + 3 < NCHUNK else []):
                    indma[c] = nc.sync.dma_start(out=xc[c][:], in_=x_d[c])
                    if c == 2:
                        # anchor c2's dispatch mid-warmup (~11.4us): late
                        # enough that c0 and c1 each get a clean bandwidth
                        # window, early enough to land before phase 2
                        _chain(indma[c], warm[6], "c2 mid-warmup")
                    else:
                        _chain(indma[c], indma[c - 2],
                               f"input c{c} after c{c - 2}")
                T = 2 * (m + 1)
                # separate per-engine output buffers so the two drain
                # engines never serialize on a shared tile: even rb ->
                # Vector (slots 0..7), odd rb -> Scalar (slots 8..15)
                dvb = dvbs[m % 3]
                acb = acbs[m % 3]
                fgate = indma[m + 2] if m + 2 < NCHUNK else None
                for rb in range(T):
                    q, r = rb // 2, rb % 2
                    # diag trim: the second diagonal tile (perm-row 2 of
                    # chunk m) only needs perm-cols 256:512
                    diag2 = (q == m and r == 1)
                    c0 = 256 if diag2 else 0
                    w = CW - c0
                    ps = pss[gctr % 7]
                    gctr += 1
                    for k in range(KT):
                        nc.tensor.matmul(
                            ps[:, 0:w],
                            xc[q][:, k, 2 * r * 128:(2 * r + 1) * 128],
                            xc[m][:, k, c0:CW],
                            start=(k == 0), stop=(k == KT - 1),
                        )
                    # stream output in <=2-slot pieces as slots complete,
                    # so only a small piece flushes after the last drain
                    sl = rb // 2
                    if m == NCHUNK - 1 and sl >= 6:
                        s0, fire, pw = sl, True, 1    # small final flushes
                    else:
                        s0, fire, pw = sl - 1, sl % 2 == 1, 2
                    if rb % 2 == 0:
                        nc.vector.tensor_scalar(
                            dvb[:, rb // 2, c0:CW], ps[:, 0:w], OSCALE,
                            nthr_t[:], op0=Alu.mult, op1=Alu.add)
                        if fire:
                            fd = nc.sync.dma_start(
                                out=d_d.ap()[m][:, s0:s0 + pw, :],
                                in_=dvb[:, s0:s0 + pw, :])
                            if fgate is not None:
                                _chain(fd, fgate, f"f{m} after c{m + 2}")
                    else:
                        nc.scalar.activation(
                            acb[:, rb // 2, c0:CW], ps[:, 0:w], Act.Identity,
                            bias=nthr_t[:], scale=OSCALE)
                        if fire:
                            if m == NCHUNK - 1 and sl == 7:
                                # ONLY the final flush rides the Scalar
                                # queue (no cross-queue hop after the last
                                # drain); earlier late-phase flushes go via
                                # Sync so their 592ns dispatches never sit
                                # in front of the final drain on Scalar.
                                # last flush: only cols 256:512 are real
                                nc.scalar.dma_start(
                                    out=d_d.ap()[m][:, 8 + s0:9 + s0,
                                                    256:],
                                    in_=acb[:, s0:s0 + 1, 256:])
                            else:
                                fd = nc.sync.dma_start(
                                    out=d_d.ap()[m][:, 8 + s0:8 + s0 + pw, :],
                                    in_=acb[:, s0:s0 + pw, :])
                                if fgate is not None:
                                    _chain(fd, fgate, f"f{m} after c{m + 2}")
                if m == 0:
                    # clock-hold: re-stream p0's diag1 into the warm bank
                    # (real 512-wide work, never drained). Duration 681ns
                    # <= the minimum observed p0->p1 input wait (0.66us),
                    # so it never delays phase 1 but keeps the HAM clock
                    # from dropping during the wait for chunk 1.
                    for k in range(KT):
                        nc.tensor.matmul(wps[:], xc[0][:, k, 0:128],
                                         xc[0][:, k, :],
                                         start=(k == 0), stop=(k == KT - 1))
                    # second hold (diag2 re-stream, +363ns -> 1.04us total):
                    # still under the typical 1.5-2.5us c1 wait, covering
                    # more of the idle before phase 1
                    for k in range(KT):
                        nc.tensor.matmul(wps[:, 0:256],
                                         xc[0][:, k, 256:384],
                                         xc[0][:, k, 256:512],
                                         start=(k == 0), stop=(k == KT - 1))
                if m < NCHUNK - 1 and m % 2 == 0:
                    # both sides have slots 0..m (odd count): flush slot m
                    fd = nc.sync.dma_start(
                        out=d_d.ap()[m][:, m:m + 1, :], in_=dvb[:, m:m + 1, :])
                    if fgate is not None:
                        _chain(fd, fgate, f"f{m}x after c{m + 2}")
                    fd = nc.sync.dma_start(
                        out=d_d.ap()[m][:, 8 + m:8 + m + 1, :],
                        in_=acb[:, m:m + 1, :])
                    if fgate is not None:
                        _chain(fd, fgate, f"f{m}x after c{m + 2}")
    nc.compile()
    return nc


def get_nc():
    global _compiled_nc
    if _compiled_nc is None:
        _compiled_nc = _build_nc()
    return _compiled_nc


def _round_mant(x, bits):
    """Round fp32 array to `bits` explicit mantissa bits."""
    m, e = np.frexp(x)
    s = np.float32(1 << bits)
    m = np.round(m * s) / s
    return np.ldexp(m, e).astype(np.float32)


# self-inverse in-chunk block perms: core h computes perm-rows {0,2}
# (orig rows {0,2} / {1,3}) of each chunk
_PERM = ([0, 1, 2, 3], [1, 0, 3, 2])

_state = {}


def make_inputs(x):
    xs = np.asarray(x)[:, :, :, 0]                      # (B, C, N) fp32
    nrm = np.sqrt(np.sum(xs * xs, axis=1, keepdims=True))
    xn = xs / np.maximum(nrm, EPS)

    Nsq = float(N) * float(N)
    in_maps = []
    xn64s, thrs = [], []
    for b in range(B):
        xb64 = xn[b].astype(np.float64)
        s = xb64.sum(axis=1)
        M = xb64 @ xb64.T
        sum_g = float(s @ s)
        sum_g2 = float((M * M).sum())
        mean = (2.0 * sum_g - 2.0 * Nsq) / Nsq
        s2 = 4.0 * sum_g2 - 8.0 * sum_g + 4.0 * Nsq
        var = (s2 - Nsq * mean * mean) / (Nsq - 1.0)
        t_b = (mean + PPF_09 * np.sqrt(var) + 2.0) / 2.0
        xn64s.append(xb64)
        thrs.append(t_b)

        nthr_dev = np.full((128, 1), -t_b * OSCALE, np.float32)
        xbr = _round_mant(xn[b].astype(np.float32), MBITS)  # (C, N)
        for h in range(2):
            xloc = xbr.reshape(C, NCHUNK, 4, 128)[:, :, _PERM[h], :]
            xloc = xloc.reshape(C, N)
            xcarr = xloc.reshape(KT, 128, NCHUNK, CW).transpose(2, 1, 0, 3)
            in_maps.append({
                "xc": np.ascontiguousarray(xcarr),
                "nthr": nthr_dev,
            })
    _state["xn64"] = xn64s
    _state["thr"] = thrs
    return in_maps


def assemble(results):
    out = np.empty((2, B * N * N), np.int32)
    iota = np.arange(N, dtype=np.int32)
    neg1 = np.int32(-1)
    for b in range(B):
        dU = np.empty((N, N), np.float32)
        for h in range(2):
            dv = results[2 * b + h]["d"]      # [m, 128, slot, 512] fp8
            for m in range(NCHUNK):
                T = 2 * (m + 1)
                blk = dv[m].astype(np.float32)             # [128, 16, 512]
                if h == 1:
                    blk = blk.reshape(128, 16, 4, 128)[:, :, _PERM[1], :]
                    blk = blk.reshape(128, 16, CW)
                for rb in range(T):
                    q, r = rb // 2, rb % 2
                    a = 4 * q + _PERM[h][2 * r]
                    slot = q + 8 * r
                    # diag trim: perm-cols 256:512 only = orig col-blocks
                    # {2,3} for both perms (perms keep {2,3} in place)
                    cl = 256 if (q == m and r == 1) else 0
                    dU[a * 128:(a + 1) * 128,
                       m * CW + cl:(m + 1) * CW] = blk[:, slot, cl:]
        adjU = (dU > 0).astype(np.uint8)
        nearU = np.triu(np.abs(dU) < MARGIN * OSCALE)
        # mask garbage in never-written regions (below-diag quarters of
        # trimmed diag tiles are inside triu=False anyway, but be safe)
        ii, jj = np.nonzero(nearU)
        if ii.size:
            xn64 = _state["xn64"][b]
            g = np.einsum('ci,ci->i', xn64[:, ii], xn64[:, jj])
            adjU[ii, jj] = g > _state["thr"][b]
        adj = np.triu(adjU)
        adj += np.triu(adjU, 1).T
        src = b * N + iota
        out[0, b * N * N:(b + 1) * N * N] = np.where(
            adj, src[:, None], neg1).ravel()
        out[1, b * N * N:(b + 1) * N * N] = np.where(
            adj, src[None, :], neg1).ravel()
    return out


def kernel(x):
    from concourse.bass_utils import run_bass_kernel_spmd

    nc = get_nc()
    in_maps = make_inputs(x)
    res = run_bass_kernel_spmd(nc, in_maps, list(range(NCORES)))
    return assemble(res.results)
